# revision 9
# baseline (speedup 1.0000x reference)
"""DeepShift Conv2dShift kernel for Trainium2 (8 NeuronCores, SPMD).

Math (matches the reference):
    v  = exp2(round(clip(shift, -14, 0))) * sign(round(sign))
       = exp2(round(shift)) * round(sign)          # shift in (-10,-1), sign in (-1,1)
    x  = round_to_fixed(input)   (absorbed into bf16 quantization; see below)
    out = conv2d(x, v, stride 1, pad 1, NCHW/OIHW) + round_to_fixed(bias)

Implementation:
  - Data-parallel over batch: 32 images -> 4 per core, weights replicated.
  - Weights are exact powers of two (or 0) -> exactly representable in bf16.
    Activations are cast to bf16; matmuls run at the bf16 TensorE rate
    (1 cycle/row vs 4 for f32). The only approximation vs the reference is
    activation rounding: rel err ~2^-9 RMS, far below tolerance.
  - Conv as implicit GEMM: per (ci_block, ky, kx) a [Cin=128 x Cout=128]
    stationary weight tile multiplies a shifted window of the zero-padded
    input plane [128 part, 58*58 free]; 18 matmuls accumulate in PSUM per
    output tile of 8 rows x 58 cols (464 <= 512 PSUM bank limit). The two
    garbage columns per row (x=56,57 of the padded frame) are never stored.
  - round(x) is computed exactly (RNE, matching jnp.round) with the
    (x + 1.5*2^23) - 1.5*2^23 float32 trick; exp2 via ACT Exp(ln2*r), whose
    tiny LUT error is snapped away by the bf16 cast (2^k is exact in bf16).
  - Weight tiles are transposed [co,ci]->[ci,co] by XBAR DMA transpose
    (bf16 SBUF->SBUF), keeping the TensorE stream purely conv matmuls.
"""

import numpy as np

import concourse.bacc as bacc
import concourse.bass as bass
import concourse.mybir as mybir
import concourse.tile as tile
from concourse.bass_utils import run_bass_kernel_spmd

F32 = mybir.dt.float32
BF16 = mybir.dt.bfloat16

N_CORES = 8
B_FULL, CIN, H, W = 32, 256, 56, 56
COUT, KH, KW = 256, 3, 3
B = B_FULL // N_CORES          # images per core
HP, WP = H + 2, W + 2          # zero-padded plane
FLAT = HP * WP                 # 3364
FLAT_ALLOC = FLAT + 4          # slack: last row-group reads 2 past the end
R = 8                          # output rows per PSUM tile
NGRP = H // R                  # 7 row groups
NFREE = R * WP                 # 464 matmul free size
CB = COUT // 128               # cout blocks
CIB = CIN // 128               # cin blocks
KK = KH * KW
M_RNE = 12582912.0             # 1.5 * 2^23: (x + M) - M == round-half-even(x)
LN2 = 0.6931471805599453


def _widx(cb, cib, ky, kx):
    return ((cb * CIB + cib) * KH + ky) * KW + kx


def build_module(reps=1):
    nc = bacc.Bacc("TRN2", debug=False, target_bir_lowering=False,
                   num_devices=N_CORES)

    inp = nc.declare_dram_parameter("input", [B, CIN, H, W], F32, isOutput=False)
    shift = nc.declare_dram_parameter("shift", [COUT, CIN, KH, KW], F32, isOutput=False)
    sign = nc.declare_dram_parameter("sign", [COUT, CIN, KH, KW], F32, isOutput=False)
    bias = nc.declare_dram_parameter("bias", [COUT], F32, isOutput=False)
    out = nc.declare_dram_parameter("out", [B, COUT, H, W], F32, isOutput=True)

    with tile.TileContext(nc) as tc:
        with (
            tc.tile_pool(name="consts", bufs=1) as consts,
            tc.tile_pool(name="wstage", bufs=2) as wstage,
            tc.tile_pool(name="xstage", bufs=3) as xstage,
            tc.tile_pool(name="xpad", bufs=2) as xpad_pool,
            tc.tile_pool(name="outp", bufs=4) as out_pool,
            tc.tile_pool(name="psum", bufs=8, space="PSUM") as psum_pool,
        ):
          for _rep in range(reps):
            # all 36 stationary weight tiles, [ci, co] layout, bf16
            wt_all = consts.tile([128, CB * CIB * KK, 128], BF16)
            bias_sb = consts.tile([128, CB], F32)

            # ---- weight transform, per (cout, cin) chunk ----
            CHW = (CIN // CIB) * KK  # 1152 free elems per chunk
            for cb in range(CB):
                for cib in range(CIB):
                    sh_t = wstage.tile([128, CHW], F32)
                    sg_t = wstage.tile([128, CHW], F32)
                    nc.sync.dma_start(
                        out=sh_t,
                        in_=shift[cb * 128:(cb + 1) * 128, cib * 128:(cib + 1) * 128]
                        .rearrange("c i kh kw -> c (i kh kw)"),
                    )
                    nc.sync.dma_start(
                        out=sg_t,
                        in_=sign[cb * 128:(cb + 1) * 128, cib * 128:(cib + 1) * 128]
                        .rearrange("c i kh kw -> c (i kh kw)"),
                    )
                    # r = round(shift)  (exact RNE)
                    nc.vector.tensor_scalar(
                        out=sh_t, in0=sh_t, scalar1=M_RNE, scalar2=M_RNE,
                        op0=mybir.AluOpType.add, op1=mybir.AluOpType.subtract,
                    )
                    # e = 2^r  (bf16 cast snaps to the exact power of two);
                    # runs on ACT while DVE rounds sign in parallel
                    e_t = wstage.tile([128, CHW], BF16)
                    nc.scalar.activation(
                        out=e_t, in_=sh_t, func=mybir.ActivationFunctionType.Exp,
                        scale=LN2,
                    )
                    # s = round(sign) in {-1, 0, 1}
                    rs_t = wstage.tile([128, CHW], BF16)
                    nc.vector.tensor_scalar(
                        out=rs_t, in0=sg_t, scalar1=M_RNE, scalar2=M_RNE,
                        op0=mybir.AluOpType.add, op1=mybir.AluOpType.subtract,
                    )
                    # v = e * s, written pos-major so each kernel position is
                    # a contiguous [128,128] block for the XBAR DMA transpose
                    v2 = wstage.tile([128, KK, 128], BF16)
                    nc.vector.tensor_tensor(
                        out=v2.rearrange("p k c -> p c k"),
                        in0=e_t.rearrange("p (c k) -> p c k", k=KK),
                        in1=rs_t.rearrange("p (c k) -> p c k", k=KK),
                        op=mybir.AluOpType.mult,
                    )
                    # [co, ci] -> [ci, co] via DMA transpose per position
                    for pos in range(KK):
                        nc.sync.dma_start(
                            out=wt_all[:, _widx(cb, cib, pos // KW, pos % KW), :],
                            in_=v2[:, pos, :],
                            transpose=True,
                        )

                # b = round_to_fixed(bias) = floor(bias * 2^16) / 2^16
                bt = wstage.tile([128, 1], F32)
                nc.sync.dma_start(
                    out=bt,
                    in_=bias[cb * 128:(cb + 1) * 128].rearrange("(c o) -> c o", o=1),
                )
                # floor(z) = RNE(z - 0.5) for our value range
                nc.vector.tensor_scalar(
                    out=bt, in0=bt, scalar1=65536.0, scalar2=0.5,
                    op0=mybir.AluOpType.mult, op1=mybir.AluOpType.subtract,
                )
                nc.vector.tensor_scalar(
                    out=bt, in0=bt, scalar1=M_RNE, scalar2=M_RNE,
                    op0=mybir.AluOpType.add, op1=mybir.AluOpType.subtract,
                )
                nc.vector.tensor_scalar_mul(
                    out=bias_sb[:, cb:cb + 1], in0=bt, scalar1=1.0 / 65536.0,
                )

            # ---- input load/pad/cast ----
            def load_image(n):
                xp = xpad_pool.tile([128, CIB, FLAT_ALLOC], BF16, tag="xp")
                # Zero only the pad positions (the interior is fully
                # overwritten by the cast-copy below):
                #   flat[0:W+3]                     top row + (1,0)
                #   (r*WP + W+1, r*WP + W+2) pairs  right/left pad columns
                #   flat[(H+1)*WP:FLAT_ALLOC]       bottom row + slack
                for cib in range(CIB):
                    plane = xp[:, cib, :]
                    nc.gpsimd.memset(plane[:, 0:W + 3], 0.0)
                    pairs = plane[:, W + 1:W + 1 + (H + 1) * WP].rearrange(
                        "p (r two) -> p r two", two=WP
                    )[:, :, 0:2]
                    nc.gpsimd.memset(pairs, 0.0)
                    nc.gpsimd.memset(plane[:, (H + 1) * WP:], 0.0)
                for cib in range(CIB):
                    xs = xstage.tile([128, H * W], F32, tag="xs")
                    nc.sync.dma_start(
                        out=xs,
                        in_=inp[n, cib * 128:(cib + 1) * 128].rearrange("c h w -> c (h w)"),
                    )
                    dst = xp[:, cib, :FLAT].rearrange("p (h w) -> p h w", h=HP)
                    nc.vector.tensor_copy(
                        out=dst[:, 1:H + 1, 1:W + 1],
                        in_=xs.rearrange("p (h w) -> p h w", h=H),
                    )
                return xp

            # ---- conv main loop ----
            def emit_taps(xp, ps, g, cb, cib, first, last):
                k = 0
                for ky in range(KH):
                    for kx in range(KW):
                        base = (R * g + ky) * WP + kx
                        nc.tensor.matmul(
                            ps,
                            lhsT=wt_all[:, _widx(cb, cib, ky, kx), :],
                            rhs=xp[:, cib, base:base + NFREE],
                            start=(first and k == 0),
                            stop=(last and k == KK - 1),
                        )
                        k += 1

            def emit_tail(ps, n, g, cb):
                ob = out_pool.tile([128, R * W], F32, tag="ob")
                nc.scalar.activation(
                    out=ob.rearrange("p (h w) -> p h w", h=R),
                    in_=ps.rearrange("p (h w) -> p h w", h=R)[:, :, :W],
                    func=mybir.ActivationFunctionType.Identity,
                    bias=bias_sb[:, cb:cb + 1], scale=1.0,
                )
                nc.sync.dma_start(
                    out=out[n, cb * 128:(cb + 1) * 128, R * g:R * (g + 1), :],
                    in_=ob.rearrange("p (h w) -> p h w", h=R),
                )

            xp_cur = load_image(0)
            for n in range(B):
                xp_next = None
                for cb in range(CB):
                    if n == 0 and cb == 0:
                        # Warm-up restructure: the ci0 taps of all 7 row-
                        # groups only need the first weight chunk + first
                        # input plane, giving the PE ~12us of runway while
                        # the remaining weight chunks stream in from HBM.
                        open_ps = []
                        for g in range(NGRP):
                            ps = psum_pool.tile([128, NFREE], F32, tag="ps")
                            emit_taps(xp_cur, ps, g, cb, 0, first=True, last=False)
                            open_ps.append(ps)
                        for g in range(NGRP):
                            emit_taps(xp_cur, open_ps[g], g, cb, 1,
                                      first=False, last=True)
                            emit_tail(open_ps[g], n, g, cb)
                    else:
                        for g in range(NGRP):
                            ps = psum_pool.tile([128, NFREE], F32, tag="ps")
                            for cib in range(CIB):
                                emit_taps(xp_cur, ps, g, cb, cib,
                                          first=(cib == 0), last=(cib == CIB - 1))
                            emit_tail(ps, n, g, cb)
                    # prefetch the next image between the two cout blocks,
                    # so its DMA lands behind this image's weight/input needs
                    if cb == 0 and n + 1 < B:
                        xp_next = load_image(n + 1)
                xp_cur = xp_next

    nc.compile()
    return nc


_CACHE = {}


def _get_module():
    if "nc" not in _CACHE:
        _CACHE["nc"] = build_module()
    return _CACHE["nc"]


def kernel(input, shift, sign, bias):
    nc = _get_module()
    input = np.ascontiguousarray(input, dtype=np.float32)
    in_maps = [
        {
            "input": input[i * B:(i + 1) * B],
            "shift": shift,
            "sign": sign,
            "bias": bias,
        }
        for i in range(N_CORES)
    ]
    res = run_bass_kernel_spmd(nc, in_maps, core_ids=list(range(N_CORES)))
    return np.concatenate([res.results[i]["out"] for i in range(N_CORES)], axis=0)


# revision 10
# speedup vs baseline: 1.2101x; 1.2101x over previous
"""DeepShift Conv2dShift kernel for Trainium2 (8 NeuronCores, SPMD).

Math (matches the reference):
    v  = exp2(round(clip(shift, -14, 0))) * sign(round(sign))
       = exp2(round(shift)) * round(sign)          # shift in (-10,-1), sign in (-1,1)
    x  = round_to_fixed(input)   (absorbed into bf16 quantization; see below)
    out = conv2d(x, v, stride 1, pad 1, NCHW/OIHW) + round_to_fixed(bias)

Implementation:
  - Data-parallel over batch: 32 images -> 4 per core, weights replicated.
  - Weights are exact powers of two (or 0) -> exactly representable in bf16.
    Activations are cast to bf16; matmuls run at the bf16 TensorE rate
    (1 cycle/row vs 4 for f32). The only approximation vs the reference is
    activation rounding: rel err ~2^-9 RMS, far below tolerance.
  - Conv as implicit GEMM: per (ci_block, ky, kx) a [Cin=128 x Cout=128]
    stationary weight tile multiplies a shifted window of the zero-padded
    input plane [128 part, 58*58 free]; 18 matmuls accumulate in PSUM per
    output tile of 8 rows x 58 cols (464 <= 512 PSUM bank limit). The two
    garbage columns per row (x=56,57 of the padded frame) are never stored.
  - round(x) is computed exactly (RNE, matching jnp.round) with the
    (x + 1.5*2^23) - 1.5*2^23 float32 trick; exp2 via ACT Exp(ln2*r), whose
    tiny LUT error is snapped away by the bf16 cast (2^k is exact in bf16).
"""

import numpy as np

import concourse.bacc as bacc
import concourse.bass as bass
import concourse.mybir as mybir
import concourse.tile as tile
from concourse.bass_utils import run_bass_kernel_spmd
from concourse.masks import make_identity

F32 = mybir.dt.float32
BF16 = mybir.dt.bfloat16

N_CORES = 8
B_FULL, CIN, H, W = 32, 256, 56, 56
COUT, KH, KW = 256, 3, 3
B = B_FULL // N_CORES          # images per core
HP, WP = H + 2, W + 2          # zero-padded plane
FLAT = HP * WP                 # 3364
FLAT_ALLOC = FLAT + 4          # slack: last row-group reads 2 past the end
R = 8                          # output rows per PSUM tile
NGRP = H // R                  # 7 row groups
NFREE = R * WP                 # 464 matmul free size
CB = COUT // 128               # cout blocks
CIB = CIN // 128               # cin blocks
M_RNE = 12582912.0             # 1.5 * 2^23: (x + M) - M == round-half-even(x)
LN2 = 0.6931471805599453


def _widx(cb, cib, ky, kx):
    return ((cb * CIB + cib) * KH + ky) * KW + kx


def build_module(reps=1):
    nc = bacc.Bacc("TRN2", debug=False, target_bir_lowering=False,
                   num_devices=N_CORES)

    inp = nc.declare_dram_parameter("input", [B, CIN, H, W], F32, isOutput=False)
    shift = nc.declare_dram_parameter("shift", [COUT, CIN, KH, KW], F32, isOutput=False)
    sign = nc.declare_dram_parameter("sign", [COUT, CIN, KH, KW], F32, isOutput=False)
    bias = nc.declare_dram_parameter("bias", [COUT], F32, isOutput=False)
    out = nc.declare_dram_parameter("out", [B, COUT, H, W], F32, isOutput=True)

    with tile.TileContext(nc) as tc:
        with (
            tc.tile_pool(name="consts", bufs=1) as consts,
            tc.tile_pool(name="wstage", bufs=4) as wstage,
            tc.tile_pool(name="xstage", bufs=3) as xstage,
            tc.tile_pool(name="xpad", bufs=2) as xpad_pool,
            tc.tile_pool(name="outp", bufs=4) as out_pool,
            tc.tile_pool(name="psum", bufs=6, space="PSUM") as psum_pool,
            tc.tile_pool(name="psumT", bufs=2, space="PSUM") as psum_t_pool,
        ):
          for _rep in range(reps):
            ident = consts.tile([128, 128], BF16)
            make_identity(nc, ident)
            # all 36 stationary weight tiles, [ci, co] layout, bf16
            wt_all = consts.tile([128, CB * CIB * KH * KW, 128], BF16)
            bias_sb = consts.tile([128, CB], F32)

            # ---- weight transform + transpose, per (cout, cin) chunk ----
            CHW = (CIN // CIB) * KH * KW  # 1152 free elems per chunk
            for cb in range(CB):
                for cib in range(CIB):
                    sh_t = wstage.tile([128, CHW], F32)
                    sg_t = wstage.tile([128, CHW], F32)
                    nc.sync.dma_start(
                        out=sh_t,
                        in_=shift[cb * 128:(cb + 1) * 128, cib * 128:(cib + 1) * 128]
                        .rearrange("c i kh kw -> c (i kh kw)"),
                    )
                    nc.sync.dma_start(
                        out=sg_t,
                        in_=sign[cb * 128:(cb + 1) * 128, cib * 128:(cib + 1) * 128]
                        .rearrange("c i kh kw -> c (i kh kw)"),
                    )
                    eng = nc.vector
                    # r = round(shift)  (exact RNE)
                    eng.tensor_scalar(
                        out=sh_t, in0=sh_t, scalar1=M_RNE, scalar2=M_RNE,
                        op0=mybir.AluOpType.add, op1=mybir.AluOpType.subtract,
                    )
                    # e = 2^r  (bf16 cast snaps to the exact power of two);
                    # runs on ACT while DVE/GpSimd round sign in parallel
                    e_t = wstage.tile([128, CHW], BF16)
                    nc.scalar.activation(
                        out=e_t, in_=sh_t, func=mybir.ActivationFunctionType.Exp,
                        scale=LN2,
                    )
                    # s = round(sign) in {-1, 0, 1}
                    rs_t = wstage.tile([128, CHW], BF16)
                    eng.tensor_scalar(
                        out=rs_t, in0=sg_t, scalar1=M_RNE, scalar2=M_RNE,
                        op0=mybir.AluOpType.add, op1=mybir.AluOpType.subtract,
                    )
                    eng.tensor_mul(out=e_t, in0=e_t, in1=rs_t)

                    # transpose [co, ci] -> [ci, co] per kernel position
                    v_view = e_t.rearrange("p (c k) -> p c k", k=KH * KW)
                    for pos in range(KH * KW):
                        tp = psum_t_pool.tile([128, 128], BF16)
                        nc.tensor.transpose(tp, v_view[:, :, pos], ident)
                        nc.vector.tensor_copy(
                            out=wt_all[:, _widx(cb, cib, pos // KW, pos % KW), :],
                            in_=tp,
                        )

                # b = round_to_fixed(bias) = floor(bias * 2^16) / 2^16
                bt = wstage.tile([128, 1], F32)
                nc.sync.dma_start(
                    out=bt,
                    in_=bias[cb * 128:(cb + 1) * 128].rearrange("(c o) -> c o", o=1),
                )
                # floor(z) = RNE(z - 0.5) for our value range
                nc.vector.tensor_scalar(
                    out=bt, in0=bt, scalar1=65536.0, scalar2=0.5,
                    op0=mybir.AluOpType.mult, op1=mybir.AluOpType.subtract,
                )
                nc.vector.tensor_scalar(
                    out=bt, in0=bt, scalar1=M_RNE, scalar2=M_RNE,
                    op0=mybir.AluOpType.add, op1=mybir.AluOpType.subtract,
                )
                nc.vector.tensor_scalar_mul(
                    out=bias_sb[:, cb:cb + 1], in0=bt, scalar1=1.0 / 65536.0,
                )

            # ---- input load/pad/cast ----
            def load_image(n):
                xp = xpad_pool.tile([128, CIB, FLAT_ALLOC], BF16, tag="xp")
                # Zero only the pad positions (the interior is fully
                # overwritten by the cast-copy below):
                #   flat[0:W+3]                     top row + (1,0)
                #   (r*WP + W+1, r*WP + W+2) pairs  right/left pad columns
                #   flat[(H+1)*WP:FLAT_ALLOC]       bottom row + slack
                for cib in range(CIB):
                    plane = xp[:, cib, :]
                    nc.gpsimd.memset(plane[:, 0:W + 3], 0.0)
                    pairs = plane[:, W + 1:W + 1 + (H + 1) * WP].rearrange(
                        "p (r two) -> p r two", two=WP
                    )[:, :, 0:2]
                    nc.gpsimd.memset(pairs, 0.0)
                    nc.gpsimd.memset(plane[:, (H + 1) * WP:], 0.0)
                for cib in range(CIB):
                    xs = xstage.tile([128, H * W], F32, tag="xs")
                    nc.sync.dma_start(
                        out=xs,
                        in_=inp[n, cib * 128:(cib + 1) * 128].rearrange("c h w -> c (h w)"),
                    )
                    dst = xp[:, cib, :FLAT].rearrange("p (h w) -> p h w", h=HP)
                    nc.vector.tensor_copy(
                        out=dst[:, 1:H + 1, 1:W + 1],
                        in_=xs.rearrange("p (h w) -> p h w", h=H),
                    )
                return xp

            xp_cur = load_image(0)
            for n in range(B):
                xp = xp_cur
                xp_next = None
                def emit_taps(ps, g, cb, cib, first, last):
                    k = 0
                    for ky in range(KH):
                        for kx in range(KW):
                            base = (R * g + ky) * WP + kx
                            nc.tensor.matmul(
                                ps,
                                lhsT=wt_all[:, _widx(cb, cib, ky, kx), :],
                                rhs=xp[:, cib, base:base + NFREE],
                                start=(first and k == 0),
                                stop=(last and k == KH * KW - 1),
                            )
                            k += 1

                def emit_tail(ps, g, cb):
                    ob = out_pool.tile([128, R * W], F32, tag="ob")
                    nc.scalar.activation(
                        out=ob.rearrange("p (h w) -> p h w", h=R),
                        in_=ps.rearrange("p (h w) -> p h w", h=R)[:, :, :W],
                        func=mybir.ActivationFunctionType.Identity,
                        bias=bias_sb[:, cb:cb + 1], scale=1.0,
                    )
                    nc.sync.dma_start(
                        out=out[n, cb * 128:(cb + 1) * 128, R * g:R * (g + 1), :],
                        in_=ob.rearrange("p (h w) -> p h w", h=R),
                    )

                for cb in range(CB):
                    if cb == 1 and n + 1 < B:
                        xp_next = load_image(n + 1)
                    if n == 0 and cb == 0:
                        # Warm-up restructure: the ci0 taps of 6 row-groups
                        # only need the first weight chunk + first input
                        # plane, giving the PE ~10us of runway while the
                        # remaining weight chunks stream in from HBM.
                        open_ps = []
                        for g in range(6):
                            ps = psum_pool.tile([128, NFREE], F32, tag="ps")
                            emit_taps(ps, g, cb, 0, first=True, last=False)
                            open_ps.append(ps)
                        for g in range(6):
                            emit_taps(open_ps[g], g, cb, 1, first=False, last=True)
                            emit_tail(open_ps[g], g, cb)
                        ps = psum_pool.tile([128, NFREE], F32, tag="ps")
                        for cib in range(CIB):
                            emit_taps(ps, 6, cb, cib, first=(cib == 0),
                                      last=(cib == CIB - 1))
                        emit_tail(ps, 6, cb)
                    else:
                        for g in range(NGRP):
                            ps = psum_pool.tile([128, NFREE], F32, tag="ps")
                            for cib in range(CIB):
                                emit_taps(ps, g, cb, cib, first=(cib == 0),
                                          last=(cib == CIB - 1))
                            emit_tail(ps, g, cb)
                xp_cur = xp_next

    nc.compile()
    return nc


_CACHE = {}


def _get_module():
    if "nc" not in _CACHE:
        _CACHE["nc"] = build_module()
    return _CACHE["nc"]


def kernel(input, shift, sign, bias):
    nc = _get_module()
    input = np.ascontiguousarray(input, dtype=np.float32)
    in_maps = [
        {
            "input": input[i * B:(i + 1) * B],
            "shift": shift,
            "sign": sign,
            "bias": bias,
        }
        for i in range(N_CORES)
    ]
    res = run_bass_kernel_spmd(nc, in_maps, core_ids=list(range(N_CORES)))
    return np.concatenate([res.results[i]["out"] for i in range(N_CORES)], axis=0)
